# revision 15
# baseline (speedup 1.0000x reference)
"""Trainium2 Bass kernel for nn_DLCF_DCA (scatter_memory).

Reference computation, per sample b (B=128, S=256, H=768, K=64):
  keep_dep[s]  = (s==0) or any_k(depend[b,k] == s-1)
  keep_dpd[s]  = (s==0) or any_k(depended[b,k] == s-1)
  mult[s]      = w2 if s-1 in depended else (w1 if s-1 in depend else 0);
                 0 if s-1 in no_connect; 1 if s==0
  y1 = x * keep_dep;  y2 = x * keep_dpd;  y3 = x * mult

Strategy: pure data parallel over batch (16 samples per core, 8 cores).
The tiny per-token multiplier tables ([B, S] = 32K floats total) are
assembled on the host from the index lists; the device does the pure
memory-bound work: stream the [4096, 768] bf16 shard in (32 consecutive
token-rows per SBUF partition, so every DMA moves 6KB contiguous chunks
per partition), apply the three per-row scalars on the vector engine,
and stream the three outputs back out on three DMA queues (sync /
scalar / gpsimd) so all 16 SDMA engines stay saturated end to end.
"""

import contextlib
import os
import sys

import numpy as np

if "/opt/trn_rl_repo" not in sys.path:
    sys.path.insert(0, "/opt/trn_rl_repo")

N_CORES = 8
B, S, H, K = 128, 256, 768, 64
BL = B // N_CORES          # samples per core
ROWS = BL * S              # 4096 token-rows per core
RPP = ROWS // 128          # 32 consecutive rows per partition
ND = 4                     # DMA tiles over the row dim (read and write)
RPT = RPP // ND            # 8 row-blocks per tile (12KB/partition per DMA)

_cache = {}


def _split_multiwaits(nc, max_waits=1):
    """walrus in this container only accepts one sync-wait per instruction;
    splice extra waits onto single-wait NoOps just before the offender."""
    from concourse import mybir

    n = 0
    for func in nc.m.functions:
        for bb in func.blocks:
            insts = bb.instructions
            i = 0
            while i < len(insts):
                ins = insts[i]
                si = getattr(ins, "sync_info", None)
                if si is None or len(si.on_wait) <= max_waits:
                    i += 1
                    continue
                waits = list(si.on_wait)
                keep = waits[-max_waits:]
                extra = waits[:-max_waits]
                nops = []
                for j in range(0, len(extra), max_waits):
                    n += 1
                    nops.append(
                        mybir.InstNoOp(
                            name=f"{ins.name}-ws{n}",
                            sync_info=mybir.SyncInfo(
                                on_wait=extra[j : j + max_waits], on_update=[]
                            ),
                            bass_nofuse=True,
                            engine=ins.engine,
                            ins=[],
                            outs=[],
                        )
                    )
                si.on_wait = keep
                for k, nop in enumerate(nops):
                    insts.insert(i + k, nop)
                i += len(nops) + 1
    return n


def _build():
    import concourse.bass as bass
    import concourse.tile as tile
    from concourse import mybir

    f32 = mybir.dt.float32
    bf16 = mybir.dt.bfloat16
    mul = mybir.AluOpType.mult
    nc = bass.Bass()

    x = nc.dram_tensor("x", [ROWS, H], bf16, kind="ExternalInput")
    masks = nc.dram_tensor("masks", [128 * 3 * RPP], f32, kind="ExternalInput")
    ys = [nc.dram_tensor(f"y{i}", [ROWS, H], bf16, kind="ExternalOutput")
          for i in (1, 2, 3)]

    with tile.TileContext(nc) as tc, contextlib.ExitStack() as ctx:
        const = ctx.enter_context(tc.tile_pool(name="const", bufs=1))
        xpool = ctx.enter_context(tc.tile_pool(name="xpool", bufs=ND))
        ypools = [
            ctx.enter_context(tc.tile_pool(name=f"y{i}p", bufs=3))
            for i in (1, 2, 3)
        ]

        # per-row multipliers, in [partition, row-in-partition] layout
        mt = const.tile([128, 3 * RPP], f32, name="masks")
        nc.scalar.dma_start(out=mt[:], in_=masks.rearrange("(p c) -> p c", p=128))

        # row = p*32 + d*4 + g: partition p owns 32 consecutive token-rows;
        # tile d moves 4 of them (6KB contiguous per partition).
        xr = x.rearrange("(p d q) h -> d p (q h)", p=128, d=ND)
        yr = [y.rearrange("(p d q) h -> d p (q h)", p=128, d=ND) for y in ys]

        xts = []
        for d in range(ND):
            t = xpool.tile([128, RPT * H], bf16, name="xt")
            nc.sync.dma_start(out=t[:], in_=xr[d])
            xts.append(t)

        # Gate compute (and so all writes) on completion of every read, via a
        # real data dependency: acc = x*0+1 per read tile, then the working
        # mask tile mm = masks * acc. This keeps the DMA timeline
        # phase-separated — pure-read burst, then pure-write burst — avoiding
        # the HBM read/write turnaround penalty (~13% per-engine throughput
        # when mixed). mm is bf16 so the big multiplies run in DVE 4x mode.
        add = mybir.AluOpType.add
        acc = const.tile([128, 1], f32, name="acc")
        for t in xts:
            nc.vector.tensor_scalar(acc[:], t[:, :1], 0.0, 1.0,
                                    op0=mul, op1=add)
        mm = const.tile([128, 3 * RPP], f32, name="mm")
        nc.vector.tensor_scalar(mm[:], mt[:], acc[:, 0:1], None, op0=mul)
        m = [mm[:, i * RPP : (i + 1) * RPP] for i in range(3)]

        # HWDGE queues only (sync + scalar): the gpsimd SWDGE path generates
        # descriptors via SBUF rings that DVE's back-to-back perf-mode ops
        # lock it out of, which serializes its writes behind the compute.
        rings = [nc.sync, nc.scalar]
        nring = 0
        for d in range(ND):
            for yi in range(3):
                yt = ypools[yi].tile([128, RPT * H], bf16, name=f"y{yi}t")
                for g in range(RPT):
                    r = d * RPT + g
                    blk = slice(g * H, (g + 1) * H)
                    nc.vector.tensor_scalar(
                        yt[:, blk], xts[d][:, blk], m[yi][:, r : r + 1],
                        None, op0=mul,
                    )
                rings[nring % 2].dma_start(out=yr[yi][d], in_=yt[:])
                nring += 1

    _split_multiwaits(nc)
    return nc


def _prep_inputs(bert_local_out, depend, depended, no_connect,
                 depend_weight, depended_weight):
    import ml_dtypes

    x = np.ascontiguousarray(
        np.asarray(bert_local_out, dtype=np.float32).astype(ml_dtypes.bfloat16)
    )
    dep = np.asarray(depend, dtype=np.int64)
    dpd = np.asarray(depended, dtype=np.int64)
    noc = np.asarray(no_connect, dtype=np.int64)
    w1 = np.asarray(depend_weight, dtype=np.float32)
    w2 = np.asarray(depended_weight, dtype=np.float32)

    # Per-token multipliers, matching the reference's scatter order exactly.
    # Index lists hold values in [0, S); position idx+1 is affected (idx=-1
    # padding or idx=S-1 land in slots 0/S which are overwritten/cropped).
    rr = np.arange(B)[:, None]
    m1 = np.zeros((B, S + 1), np.float32)
    m1[rr, dep + 1] = 1.0
    m2 = np.zeros((B, S + 1), np.float32)
    m2[rr, dpd + 1] = 1.0
    m3 = np.zeros((B, S + 1), np.float32)
    m3[rr, dep + 1] = np.broadcast_to(w1[:, None], (B, K))
    m3[rr, dpd + 1] = np.broadcast_to(w2[:, None], (B, K))
    m3[rr, noc + 1] = 0.0
    for mm in (m1, m2, m3):
        mm[:, 0] = 1.0
    masks = np.stack([m1[:, :S], m2[:, :S], m3[:, :S]])  # [3, B, S]

    in_maps = []
    for c in range(N_CORES):
        sl = slice(c * BL, (c + 1) * BL)
        mc = masks[:, sl].reshape(3, 128, RPP)          # row = p*32 + r
        mc = np.ascontiguousarray(mc.transpose(1, 0, 2))  # [128, 3, RPP]
        in_maps.append({
            "x": x[sl].reshape(ROWS, H),
            "masks": mc.reshape(-1),
        })
    return in_maps


def kernel(bert_local_out, depend, depended, no_connect,
           depend_weight, depended_weight):
    from concourse.bass_utils import run_bass_kernel_spmd

    if "nc" not in _cache:
        _cache["nc"] = _build()
    nc = _cache["nc"]

    in_maps = _prep_inputs(bert_local_out, depend, depended, no_connect,
                           depend_weight, depended_weight)

    pdir = os.environ.get("KERNEL_PROFILE_DIR")
    ctx = contextlib.nullcontext()
    if pdir:
        import concourse.bass2jax as b2j
        from trn_agent_boot.trn_boot import _ntff_profile_via_ctypes

        if not getattr(b2j, "_neff_capture_patched", False):
            orig = b2j.rename_neff_tensors_and_patch_header

            def patched(neff_path, mapping):
                data = orig(neff_path, mapping)
                cap = os.environ.get("KERNEL_PROFILE_DIR")
                if cap:
                    os.makedirs(cap, exist_ok=True)
                    with open(os.path.join(cap, "model.neff"), "wb") as f:
                        f.write(data)
                return data

            b2j.rename_neff_tensors_and_patch_header = patched
            b2j._neff_capture_patched = True
        os.makedirs(pdir, exist_ok=True)
        hookf = _ntff_profile_via_ctypes("/opt/axon/libaxon_pjrt.so")
        if hookf is not None:
            dev = None if os.environ.get("KERNEL_PROFILE_ALL") else [0]
            ctx = hookf(pdir, dev)

    with ctx:
        res = run_bass_kernel_spmd(nc, in_maps, list(range(N_CORES)))

    outs = []
    for name in ("y1", "y2", "y3"):
        full = np.empty((B, S, H), dtype=np.float32)
        for c in range(N_CORES):
            full[c * BL : (c + 1) * BL] = (
                res.results[c][name].astype(np.float32).reshape(BL, S, H)
            )
        outs.append(full)
    return tuple(outs)
